# revision 1
# baseline (speedup 1.0000x reference)
"""Trainium2 Bass kernel for CLIPAttention (B=32, S=512, E=768, H=12, D=64).

Strategy: data-parallel over batch across 8 NeuronCores (4 batches/core).
All matmul operands are fp16 (PSUM accumulates fp32); fp16 stationary
operands get fast-weight-load, which fp32/f32r weights do not.

hidden_states and weights are pre-cast to fp16 on the host (identical
rounding to an on-chip cast, half the DMA bytes, no staging copies).

Per batch:
  x(fp16) -> xT (PE transpose) -> qT/kT feature-major + v token-major
  projections. Attention per head, with scores computed TRANSPOSED (k-major)
  so no transpose of the probabilities is ever needed:
    scoresT[k,q] = kh.T @ qh    (PE, triangular: only blocks with k <= q)
    pE = exp(scale * scoresT)   (ACT, written straight to SBUF as fp16)
    diagonal block masked by multiplying with an upper-triangular 0/1 tile
    den[q] = ones.T @ pE        (PE matmuls accumulating over k-tiles)
    po = v_h.T @ pE             (PE, triangular; unnormalized - the per-q
                                 normalization factors out of the k-sum)
    rden = approx-recip(den)    (DVE) -> broadcast to 128 partitions (GPSIMD)
    outT copy = po * rden       (DVE, fused into the PSUM->SBUF copy;
                                 partition-shifted writes put odd heads at
                                 partitions 64:127 directly)
  Final projection back to token-major; biases folded into PSUM->SBUF copies.
"""

import os
import time

import numpy as np
from contextlib import ExitStack

import concourse.bass as bass
import concourse.mybir as mybir
import concourse.tile as tile
from concourse import bacc
from concourse.bass_utils import run_bass_kernel_spmd
from concourse.masks import make_identity, make_upper_triangular

B, S, E, H, D = 32, 512, 768, 12, 64
NCORES = 8
NB = B // NCORES          # batches per core
P = 128
KT = E // P               # 6 feature tiles
QT = S // P               # 4 token tiles
SCALE = float(D) ** -0.5  # 0.125
F32 = mybir.dt.float32
F16 = mybir.dt.float16

AF = mybir.ActivationFunctionType
OP = mybir.AluOpType


def _build():
    nc = bacc.Bacc(trn_type="TRN2")

    hs = nc.dram_tensor("hs", [NB, S, E], F16, kind="ExternalInput")
    w_dr = {}
    b_dr = {}
    for nm in ("q", "k", "v", "o"):
        w_dr[nm] = nc.dram_tensor(f"W{nm}", [E, E], F16, kind="ExternalInput")
        b_dr[nm] = nc.dram_tensor(f"b{nm}", [E], F32, kind="ExternalInput")
    out = nc.dram_tensor("out", [NB, S, E], F32, kind="ExternalOutput")

    with ExitStack() as ctx:
        tc = ctx.enter_context(tile.TileContext(nc))

        singles = ctx.enter_context(tc.tile_pool(name="singles", bufs=1))
        x16pool = ctx.enter_context(tc.tile_pool(name="x16pool", bufs=4))
        xtpool = ctx.enter_context(tc.tile_pool(name="xtpool", bufs=2))
        qkvpool = ctx.enter_context(tc.tile_pool(name="qkvpool", bufs=2))
        pepool = ctx.enter_context(tc.tile_pool(name="pepool", bufs=6))
        rpool = ctx.enter_context(tc.tile_pool(name="rpool", bufs=4))
        otpool = ctx.enter_context(tc.tile_pool(name="otpool", bufs=2))
        opool = ctx.enter_context(tc.tile_pool(name="opool", bufs=2))

        ps_mm = ctx.enter_context(tc.tile_pool(name="ps_mm", bufs=2, space="PSUM"))
        ps_s = ctx.enter_context(tc.tile_pool(name="ps_s", bufs=3, space="PSUM"))
        ps_pv = ctx.enter_context(tc.tile_pool(name="ps_pv", bufs=2, space="PSUM"))
        ps_den = ctx.enter_context(tc.tile_pool(name="ps_den", bufs=1, space="PSUM"))

        # ---- constants ----
        ident16 = singles.tile([P, P], F16, name="ident16")
        make_identity(nc, ident16)
        # upper-triangular (incl diagonal) 0/1 mask: keeps q >= k entries of
        # a k-major diagonal block
        triu01 = singles.tile([P, P], F16, name="triu01")
        make_upper_triangular(nc, triu01, val=1.0, diag=True)
        ones16 = singles.tile([P, 1], F16, name="ones16")
        nc.vector.memset(ones16, 1.0)

        # prefetch batch 0's x tiles ahead of the bulk weight DMA so the
        # transpose pipeline starts immediately
        x16_pre = []
        for i in range(QT):
            x16 = x16pool.tile([P, E], F16, name=f"x16p_{i}", tag="x16")
            nc.sync.dma_start(out=x16, in_=hs[0, i * P:(i + 1) * P, :])
            x16_pre.append(x16)

        # weights arrive pre-cast to fp16 from the host; DMA straight in
        w_sb = {}
        for nm in ("q", "k", "v", "o"):
            w_sb[nm] = singles.tile([P, KT, E], F16, name=f"w_{nm}")
            nc.sync.dma_start(
                out=w_sb[nm], in_=w_dr[nm].rearrange("(ko p) m -> p ko m", p=P)
            )

        # per-partition bias form for feature-major q/k
        bias_pp = {}
        for nm in ("q", "k"):
            bias_pp[nm] = singles.tile([P, KT], F32, name=f"bpp_{nm}")
            nc.sync.dma_start(
                out=bias_pp[nm], in_=b_dr[nm].rearrange("(ko p) -> p ko", p=P)
            )
        # broadcast-to-all-partitions bias form for token-major v/o
        bias_bc = {}
        for nm in ("v", "o"):
            bias_bc[nm] = singles.tile([P, E], F32, name=f"bbc_{nm}")
            src = b_dr[nm][:]
            bcast = bass.AP(tensor=src.tensor, offset=src.offset, ap=[[0, P], *src.ap])
            nc.sync.dma_start(out=bias_bc[nm], in_=bcast)

        NSPLIT = 384  # N-tile for the two token-major projections (768 = 2x384)
        HN = NSPLIT // D  # heads per N-chunk group = 6

        for b in range(NB):
            # ---- stage A: load x, cast fp16, transpose to feature-major xT ----
            xt = xtpool.tile([P, KT, S], F16, name=f"xt_{b}", tag="xt")
            for i in range(QT):
                if b == 0:
                    x16 = x16_pre[i]
                else:
                    x16 = x16pool.tile([P, E], F16, name=f"x16_{b}_{i}", tag="x16")
                    nc.sync.dma_start(out=x16, in_=hs[b, i * P:(i + 1) * P, :])
                for half in range(2):
                    tpx = ps_s.tile([P, S], F16, name=f"tpx_{b}_{i}_{half}", tag="s")
                    for jj in range(3):
                        j = 3 * half + jj
                        nc.tensor.transpose(
                            tpx[:, jj * P:(jj + 1) * P],
                            x16[:, j * P:(j + 1) * P],
                            ident16,
                        )
                    nc.vector.tensor_copy(
                        out=xt[:, 3 * half:3 * half + 3, i * P:(i + 1) * P],
                        in_=tpx[:, :3 * P].rearrange("p (j c) -> p j c", c=P),
                    )

            # ---- stage B: qT, kT feature-major [768, 512] ----
            qkv = {}
            for nm in ("q", "k"):
                dst = qkvpool.tile([P, KT, S], F16, name=f"{nm}T_{b}", tag=f"{nm}T")
                qkv[nm] = dst
                for m in range(KT):
                    ps = ps_mm.tile([P, S], F32, name=f"ps{nm}_{b}_{m}", tag="mm")
                    for kk in range(KT):
                        nc.tensor.matmul(
                            ps,
                            lhsT=w_sb[nm][:, kk, m * P:(m + 1) * P],
                            rhs=xt[:, kk, :],
                            start=(kk == 0),
                            stop=(kk == KT - 1),
                        )
                    if m % 2 == 0:
                        nc.scalar.activation(
                            out=dst[:, m, :],
                            in_=ps,
                            func=AF.Identity,
                            bias=bias_pp[nm][:, m:m + 1],
                            scale=1.0,
                        )
                    else:
                        nc.vector.tensor_scalar_add(
                            out=dst[:, m, :],
                            in0=ps,
                            scalar1=bias_pp[nm][:, m:m + 1],
                        )

            # ---- stage C: v token-major [512, 768] ----
            v_t = qkvpool.tile([P, QT, E], F16, name=f"v_{b}", tag="v")
            for i in range(QT):
                for n in range(E // NSPLIT):
                    ps = ps_mm.tile([P, S], F32, name=f"psv_{b}_{i}_{n}", tag="mm")
                    for kk in range(KT):
                        nc.tensor.matmul(
                            ps[:, :NSPLIT],
                            lhsT=xt[:, kk, i * P:(i + 1) * P],
                            rhs=w_sb["v"][:, kk, n * NSPLIT:(n + 1) * NSPLIT],
                            start=(kk == 0),
                            stop=(kk == KT - 1),
                        )
                    nc.vector.tensor_tensor(
                        out=v_t[:, i, n * NSPLIT:(n + 1) * NSPLIT],
                        in0=ps[:, :NSPLIT],
                        in1=bias_bc["v"][:, n * NSPLIT:(n + 1) * NSPLIT],
                        op=OP.add,
                    )

            # ---- stage D: attention heads (k-major probs, no transposes) ----
            # software-pipelined by one head: head h+1's scores/exp are
            # emitted before head h's PV so the PE always has independent
            # matmul work while the exp chain runs
            outT = otpool.tile([P, KT, S], F16, name=f"outT_{b}", tag="outT")
            pE_live = {}

            def emit_scores(h):
                g, rr = h // 2, h % 2
                qh = qkv["q"][rr * D:(rr + 1) * D, g, :]
                kh = qkv["k"][rr * D:(rr + 1) * D, g, :]
                pE = pepool.tile([P, QT, S], F16, name=f"pE_{b}_{h}", tag="pE")
                pE_live[h] = pE
                for j in range(QT):
                    q0 = j * P
                    n_mm = S - q0
                    ps = ps_s.tile([P, S], F32, name=f"pss_{b}_{h}_{j}", tag="s")
                    nc.tensor.matmul(
                        ps[:, :n_mm],
                        lhsT=kh[:, j * P:(j + 1) * P],
                        rhs=qh[:, q0:],
                        start=True,
                        stop=True,
                    )
                    nc.scalar.activation(
                        out=pE[:, j, q0:],
                        in_=ps[:, :n_mm],
                        func=AF.Exp,
                        scale=SCALE,
                    )
                    # causal mask on the diagonal block: keep q >= k
                    nc.vector.tensor_tensor(
                        out=pE[:, j, q0:q0 + P],
                        in0=pE[:, j, q0:q0 + P],
                        in1=triu01,
                        op=OP.mult,
                    )

            def emit_pv(h):
                g, rr = h // 2, h % 2
                pE = pE_live.pop(h)
                # denominator: ones.T @ pE accumulated over k-tiles
                den = ps_den.tile([1, S], F32, name=f"den_{b}_{h}", tag="den")
                for j in range(QT):
                    nc.tensor.matmul(
                        den[:, j * P:],
                        lhsT=ones16,
                        rhs=pE[:, j, j * P:],
                        start=(j == 0),
                        stop=(j == QT - 1),
                        skip_group_check=True,
                    )
                # PV, unnormalized, triangular over valid k-ranges
                po = ps_pv.tile([D, S], F32, name=f"po_{b}_{h}", tag="pv")
                for j in range(QT):
                    nc.tensor.matmul(
                        po[:, j * P:],
                        lhsT=v_t[:, j, h * D:(h + 1) * D],
                        rhs=pE[:, j, j * P:],
                        start=(j == 0),
                        stop=(j == QT - 1),
                        skip_group_check=True,
                    )
                rden = rpool.tile([1, S], F32, name=f"rden_{b}_{h}", tag="rden")
                nc.vector.reciprocal_approx_fast(rden, den)
                rb = rpool.tile([P, S], F32, name=f"rb_{b}_{h}", tag="rb")
                nc.gpsimd.partition_broadcast(rb, rden)
                # normalization fused into the PSUM->SBUF copy; the write is
                # partition-shifted for odd heads (engines support src/dst
                # partition bases differing)
                nc.vector.tensor_tensor(
                    out=outT[rr * D:(rr + 1) * D, g, :],
                    in0=po,
                    in1=rb[0:D, :],
                    op=OP.mult,
                )

            for h in range(H + 1):
                if h < H:
                    emit_scores(h)
                if h >= 1:
                    emit_pv(h - 1)

            # ---- stage E: final projection, token-major out ----
            for i in range(QT):
                o_t = opool.tile([P, E], F32, name=f"o_{b}_{i}", tag="o")
                for n in range(E // NSPLIT):
                    ps = ps_mm.tile([P, S], F32, name=f"pso_{b}_{i}_{n}", tag="mm")
                    for kk in range(KT):
                        nc.tensor.matmul(
                            ps[:, :NSPLIT],
                            lhsT=outT[:, kk, i * P:(i + 1) * P],
                            rhs=w_sb["o"][:, kk, n * NSPLIT:(n + 1) * NSPLIT],
                            start=(kk == 0),
                            stop=(kk == KT - 1),
                        )
                    nc.vector.tensor_tensor(
                        out=o_t[:, n * NSPLIT:(n + 1) * NSPLIT],
                        in0=ps[:, :NSPLIT],
                        in1=bias_bc["o"][:, n * NSPLIT:(n + 1) * NSPLIT],
                        op=OP.add,
                    )
                nc.sync.dma_start(out=out[b, i * P:(i + 1) * P, :], in_=o_t)

    nc.compile()
    return nc


_NC_CACHE = None


def _get_nc():
    global _NC_CACHE
    if _NC_CACHE is None:
        _NC_CACHE = _build()
    return _NC_CACHE


def run(inputs, trace=False):
    if trace:
        os.environ.pop("BASS_NEVER_TRACE", None)
    else:
        # keep the spmd runner off the NTFF trace path (the profiling hook
        # module is not always present)
        os.environ["BASS_NEVER_TRACE"] = "1"
    # hidden_states and weights are pre-cast to fp16 on the host: identical
    # rounding to the on-chip cast, but half the DMA bytes and no staging
    hs = np.ascontiguousarray(
        np.asarray(inputs["hidden_states"], dtype=np.float32).astype(np.float16)
    )
    assert hs.shape == (B, S, E)
    wb = {}
    for nm in ("q", "k", "v", "o"):
        wb[f"W{nm}"] = np.ascontiguousarray(
            np.asarray(inputs[f"W{nm}"], dtype=np.float32).astype(np.float16)
        )
        wb[f"b{nm}"] = np.ascontiguousarray(
            np.asarray(inputs[f"b{nm}"], dtype=np.float32)
        )

    nc = _get_nc()
    in_maps = []
    for c in range(NCORES):
        m = {"hs": hs[c * NB:(c + 1) * NB]}
        m.update(wb)
        in_maps.append(m)
    res = run_bass_kernel_spmd(
        nc, in_maps, core_ids=list(range(NCORES)), trace=trace
    )
    outp = np.concatenate([r_["out"] for r_ in res.results], axis=0)
    return outp, res


def kernel(**inputs) -> np.ndarray:
    # retry once on transient accelerator errors (rare NRT exec glitches)
    last = None
    for attempt in range(2):
        try:
            outp, _ = run(inputs, trace=False)
            return outp
        except Exception as e:  # noqa: BLE001
            last = e
            time.sleep(10)
    raise last



# revision 8
# speedup vs baseline: 1.2284x; 1.2284x over previous
"""Trainium2 Bass kernel for CLIPAttention (B=32, S=512, E=768, H=12, D=64).

Strategy: data-parallel over batch across 8 NeuronCores (4 batches/core).
All matmul operands are fp16 (PSUM accumulates fp32); hidden_states and
weights are pre-cast to fp16 on the host.

Two key structural optimizations over the straightforward per-batch version:

1. Denominator folding: v is stored augmented with a ones column
   (v_aug[k, h, 0:64] = v, v_aug[k, h, 64] = 1), so the PV matmul
   po = v_aug.T @ pE produces the softmax denominator in row 64 for free.
   This removes all separate ones.T @ pE denominator matmuls (~34us of PE
   time) from the kernel.

2. Cross-batch software pipelining: the PE executes its queue in order and
   drops to a lower clock p-state after any idle gap, so attention's
   exp-latency stalls are doubly expensive.  Batch b's attention (scores ->
   exp -> PV, head-pipelined 2-3 deep) is emitted interleaved with batch
   b+1's projection matmuls as filler, so the PE always has independent
   work between a head's scores and its PV (which must wait for the exp
   chain on the Scalar engine).  Filler units are chunked in sequential
   order (never reordered) so the Tile dependency tracker sees reads after
   the writes they depend on.

Attention itself: scores are computed TRANSPOSED (k-major) so no transpose
of the probabilities is needed:
    scoresT[k,q] = kh.T @ qh    (PE, triangular: only blocks with k <= q)
    pE = exp(scale * scoresT)   (ACT, written straight to SBUF as fp16)
    diagonal block masked by multiplying with an upper-triangular 0/1 tile
    po = v_aug.T @ pE           (PE, triangular; row 64 = denominator)
    rden = approx-recip(po[64]) (DVE) -> broadcast to 64 partitions (GPSIMD)
    outT copy = po[0:64] * rden (DVE, fused into the PSUM->SBUF copy;
                                 partition-shifted writes put odd heads at
                                 partitions 64:127 directly)
Final projection back to token-major; biases folded into PSUM->SBUF copies.
"""

import os
import time

import numpy as np
from contextlib import ExitStack
from functools import partial

import concourse.bass as bass
import concourse.mybir as mybir
import concourse.tile as tile
from concourse import bacc
from concourse.bass_utils import run_bass_kernel_spmd
from concourse.masks import make_identity, make_upper_triangular

B, S, E, H, D = 32, 512, 768, 12, 64
NCORES = 8
NB = B // NCORES          # batches per core
P = 128
KT = E // P               # 6 feature tiles
QT = S // P               # 4 token tiles
DV = D + 1                # v feature dim augmented with a ones column
SCALE = float(D) ** -0.5  # 0.125
F32 = mybir.dt.float32
F16 = mybir.dt.float16

AF = mybir.ActivationFunctionType
OP = mybir.AluOpType

NSPLIT = 384              # N-tile for the two token-major projections
NCH = E // NSPLIT         # 2 chunks


def _build():
    nc = bacc.Bacc(trn_type="TRN2")

    hs = nc.dram_tensor("hs", [NB, S, E], F16, kind="ExternalInput")
    w_dr = {}
    b_dr = {}
    for nm in ("q", "k", "v", "o"):
        w_dr[nm] = nc.dram_tensor(f"W{nm}", [E, E], F16, kind="ExternalInput")
        b_dr[nm] = nc.dram_tensor(f"b{nm}", [E], F32, kind="ExternalInput")
    out = nc.dram_tensor("out", [NB, S, E], F32, kind="ExternalOutput")

    with ExitStack() as ctx:
        tc = ctx.enter_context(tile.TileContext(nc))

        singles = ctx.enter_context(tc.tile_pool(name="singles", bufs=1))
        x16pool = ctx.enter_context(tc.tile_pool(name="x16pool", bufs=8))
        xtpool = ctx.enter_context(tc.tile_pool(name="xtpool", bufs=2))
        qkvpool = ctx.enter_context(tc.tile_pool(name="qkvpool", bufs=2))
        pepool = ctx.enter_context(tc.tile_pool(name="pepool", bufs=6))
        rpool = ctx.enter_context(tc.tile_pool(name="rpool", bufs=4))
        otpool = ctx.enter_context(tc.tile_pool(name="otpool", bufs=2))
        opool = ctx.enter_context(tc.tile_pool(name="opool", bufs=4))

        # PSUM budget (8 banks): scores 4 + shared mm/transpose 2 + pv 2.
        # Scores get a full head's worth of banks so the j2/j3 matmuls never
        # wait on the exp of j0/j1 (ring reuse is one head later instead).
        ps_s = ctx.enter_context(tc.tile_pool(name="ps_s", bufs=4, space="PSUM"))
        ps_mm = ctx.enter_context(tc.tile_pool(name="ps_mm", bufs=2, space="PSUM"))
        ps_pv = ctx.enter_context(tc.tile_pool(name="ps_pv", bufs=2, space="PSUM"))

        # ---- constants ----
        ident16 = singles.tile([P, P], F16, name="ident16")
        make_identity(nc, ident16)
        # upper-triangular (incl diagonal) 0/1 mask: keeps q >= k entries of
        # a k-major diagonal block
        triu01 = singles.tile([P, P], F16, name="triu01")
        make_upper_triangular(nc, triu01, val=1.0, diag=True)

        # prefetch batch 0's x tiles ahead of the bulk weight DMA so the
        # transpose pipeline starts immediately
        x16_pre = []
        for i in range(QT):
            x16 = x16pool.tile([P, E], F16, name=f"x16p_{i}", tag="x16")
            nc.sync.dma_start(out=x16, in_=hs[0, i * P:(i + 1) * P, :])
            x16_pre.append(x16)

        # weights arrive pre-cast to fp16 from the host; DMA straight in
        w_sb = {}
        for nm in ("q", "k", "v", "o"):
            w_sb[nm] = singles.tile([P, KT, E], F16, name=f"w_{nm}")
            nc.sync.dma_start(
                out=w_sb[nm], in_=w_dr[nm].rearrange("(ko p) m -> p ko m", p=P)
            )

        # per-partition bias form for feature-major q/k
        bias_pp = {}
        for nm in ("q", "k"):
            bias_pp[nm] = singles.tile([P, KT], F32, name=f"bpp_{nm}")
            nc.sync.dma_start(
                out=bias_pp[nm], in_=b_dr[nm].rearrange("(ko p) -> p ko", p=P)
            )
        # broadcast-to-all-partitions bias form for token-major v/o
        bias_bc = {}
        for nm in ("v", "o"):
            bias_bc[nm] = singles.tile([P, E], F32, name=f"bbc_{nm}")
            src = b_dr[nm][:]
            bcast = bass.AP(tensor=src.tensor, offset=src.offset, ap=[[0, P], *src.ap])
            nc.sync.dma_start(out=bias_bc[nm], in_=bcast)

        # ---- per-batch live tiles ----
        xt_t = {}       # feature-major x [P, KT, S]
        qk_t = {}       # (b, nm) -> feature-major projection [P, KT, S]
        va_t = {}       # augmented v [P, QT, H, DV] (col D is ones)
        ot_t = {}       # feature-major attention output [P, KT, S]
        o_live = {}     # (b, i) -> final output tile [P, E]
        pE_live = {}    # (b, h) -> exp'd probabilities [P, QT, S]
        pending_norm = []  # deferred PSUM->SBUF normalization copies

        # ---- stage units (each emits one PE chunk + its consumers) ----

        def unit_A(b, i):
            # load x tile i, transpose to feature-major xt
            if b == 0:
                x16 = x16_pre[i]
            else:
                x16 = x16pool.tile([P, E], F16, name=f"x16_{b}_{i}", tag="x16")
                nc.sync.dma_start(out=x16, in_=hs[b, i * P:(i + 1) * P, :])
            if i == 0:
                xt_t[b] = xtpool.tile([P, KT, S], F16, name=f"xt_{b}", tag="xt")
            xt = xt_t[b]
            # all 6 transposed 128x128 fp16 blocks fit in one PSUM bank;
            # shares the projection psum ring (tag "mm")
            tpx = ps_mm.tile([P, E], F16, name=f"tpx_{b}_{i}", tag="mm")
            for j in range(KT):
                nc.tensor.transpose(
                    tpx[:, j * P:(j + 1) * P],
                    x16[:, j * P:(j + 1) * P],
                    ident16,
                )
            nc.vector.tensor_copy(
                out=xt[:, :, i * P:(i + 1) * P],
                in_=tpx.rearrange("p (j c) -> p j c", c=P),
            )

        def unit_B(b, nm, m):
            # q/k feature-major projection, output tile m
            if (b, nm) not in qk_t:
                qk_t[(b, nm)] = qkvpool.tile(
                    [P, KT, S], F16, name=f"{nm}T_{b}", tag=f"{nm}T"
                )
            dst = qk_t[(b, nm)]
            xt = xt_t[b]
            ps = ps_mm.tile([P, S], F32, name=f"ps{nm}_{b}_{m}", tag="mm")
            for kk in range(KT):
                nc.tensor.matmul(
                    ps,
                    lhsT=w_sb[nm][:, kk, m * P:(m + 1) * P],
                    rhs=xt[:, kk, :],
                    start=(kk == 0),
                    stop=(kk == KT - 1),
                )
            if m % 2 == 0:
                nc.scalar.activation(
                    out=dst[:, m, :],
                    in_=ps,
                    func=AF.Identity,
                    bias=bias_pp[nm][:, m:m + 1],
                    scale=1.0,
                )
            else:
                nc.vector.tensor_scalar_add(
                    out=dst[:, m, :],
                    in0=ps,
                    scalar1=bias_pp[nm][:, m:m + 1],
                )

        def unit_C(b, i, n):
            # v token-major projection into the augmented layout
            if b not in va_t:
                va_t[b] = qkvpool.tile([P, QT, H, DV], F16, name=f"v_{b}", tag="v")
                nc.vector.memset(va_t[b][:, :, :, D:DV], 1.0)
            va = va_t[b]
            xt = xt_t[b]
            hn = NSPLIT // D  # heads per chunk
            ps = ps_mm.tile([P, S], F32, name=f"psv_{b}_{i}_{n}", tag="mm")
            for kk in range(KT):
                nc.tensor.matmul(
                    ps[:, :NSPLIT],
                    lhsT=xt[:, kk, i * P:(i + 1) * P],
                    rhs=w_sb["v"][:, kk, n * NSPLIT:(n + 1) * NSPLIT],
                    start=(kk == 0),
                    stop=(kk == KT - 1),
                )
            nc.vector.tensor_tensor(
                out=va[:, i, n * hn:(n + 1) * hn, 0:D],
                in0=ps[:, :NSPLIT].rearrange("p (h d) -> p h d", d=D),
                in1=bias_bc["v"][:, n * NSPLIT:(n + 1) * NSPLIT].rearrange(
                    "p (h d) -> p h d", d=D
                ),
                op=OP.add,
            )

        def unit_E(b, i, n):
            # final projection back to token-major; DMA out after last chunk
            if (b, i) not in o_live:
                o_live[(b, i)] = opool.tile([P, E], F32, name=f"o_{b}_{i}", tag="o")
            o_t = o_live[(b, i)]
            outT = ot_t[b]
            ps = ps_mm.tile([P, S], F32, name=f"pso_{b}_{i}_{n}", tag="mm")
            for kk in range(KT):
                nc.tensor.matmul(
                    ps[:, :NSPLIT],
                    lhsT=outT[:, kk, i * P:(i + 1) * P],
                    rhs=w_sb["o"][:, kk, n * NSPLIT:(n + 1) * NSPLIT],
                    start=(kk == 0),
                    stop=(kk == KT - 1),
                )
            nc.vector.tensor_tensor(
                out=o_t[:, n * NSPLIT:(n + 1) * NSPLIT],
                in0=ps[:, :NSPLIT],
                in1=bias_bc["o"][:, n * NSPLIT:(n + 1) * NSPLIT],
                op=OP.add,
            )
            if n == NCH - 1:
                nc.sync.dma_start(out=out[b, i * P:(i + 1) * P, :], in_=o_t)
                del o_live[(b, i)]

        # ---- attention ----

        def emit_scores(b, h):
            g, rr = h // 2, h % 2
            qh = qk_t[(b, "q")][rr * D:(rr + 1) * D, g, :]
            kh = qk_t[(b, "k")][rr * D:(rr + 1) * D, g, :]
            pE = pepool.tile([P, QT, S], F16, name=f"pE_{b}_{h}", tag="pE")
            pE_live[(b, h)] = pE
            for j in range(QT):
                q0 = j * P
                n_mm = S - q0
                ps = ps_s.tile([P, S], F32, name=f"pss_{b}_{h}_{j}", tag="s")
                nc.tensor.matmul(
                    ps[:, :n_mm],
                    lhsT=kh[:, j * P:(j + 1) * P],
                    rhs=qh[:, q0:],
                    start=True,
                    stop=True,
                )
                nc.scalar.activation(
                    out=pE[:, j, q0:],
                    in_=ps[:, :n_mm],
                    func=AF.Exp,
                    scale=SCALE,
                )
                # causal mask on the diagonal block: keep q >= k
                nc.vector.tensor_tensor(
                    out=pE[:, j, q0:q0 + P],
                    in0=pE[:, j, q0:q0 + P],
                    in1=triu01,
                    op=OP.mult,
                )

        def flush_norms():
            while pending_norm:
                pending_norm.pop(0)()

        def emit_pv(b, h):
            # flush the previous head's deferred normalization BEFORE this
            # head's matmuls: the ps_pv ring has 2 buffers, so a norm read
            # emitted after the ring slot's next writer would be an untracked
            # race on hardware
            flush_norms()
            g, rr = h // 2, h % 2
            pE = pE_live.pop((b, h))
            va = va_t[b]
            if h == 0:
                ot_t[b] = otpool.tile([P, KT, S], F16, name=f"outT_{b}", tag="outT")
            outT = ot_t[b]
            # PV with the ones column: row D of po is the softmax denominator
            po = ps_pv.tile([DV, S], F32, name=f"po_{b}_{h}", tag="pv")
            for j in range(QT):
                nc.tensor.matmul(
                    po[:, j * P:],
                    lhsT=va[:, j, h, :],
                    rhs=pE[:, j, j * P:],
                    start=(j == 0),
                    stop=(j == QT - 1),
                    skip_group_check=True,
                )
            # stage the denominator row to SBUF partition 0 first: the custom
            # reciprocal ucode reading PSUM at partition base 64 produced
            # corrupted columns on hardware (sim-clean, race-replay-clean)
            den_sb = rpool.tile([1, S], F32, name=f"den_{b}_{h}", tag="den")
            nc.vector.tensor_copy(out=den_sb, in_=po[D:DV, :])
            rden = rpool.tile([1, S], F32, name=f"rden_{b}_{h}", tag="rden")
            nc.vector.reciprocal_approx_fast(rden, den_sb)
            rb = rpool.tile([D, S], F32, name=f"rb_{b}_{h}", tag="rb")
            nc.gpsimd.partition_broadcast(rb, rden)

            def norm():
                # normalization fused into the PSUM->SBUF copy; the write is
                # partition-shifted for odd heads (engines support src/dst
                # partition bases differing)
                nc.vector.tensor_tensor(
                    out=outT[rr * D:(rr + 1) * D, g, :],
                    in0=po[0:D, :],
                    in1=rb,
                    op=OP.mult,
                )

            # defer the normalization copy so the DVE queue doesn't stall
            # behind the GPSIMD broadcast while masks for the next head wait
            pending_norm.append(norm)

        # ---- schedule ----

        def proj_units(b):
            units = [partial(unit_A, b, i) for i in range(QT)]
            bu = [partial(unit_B, b, nm, m) for m in range(KT) for nm in ("q", "k")]
            cu = [partial(unit_C, b, i, n) for i in range(QT) for n in range(NCH)]
            # interleave B and C (order among them is free; both only need A)
            mix = []
            bi = ci = 0
            while bi < len(bu) or ci < len(cu):
                for _ in range(3):
                    if bi < len(bu):
                        mix.append(bu[bi])
                        bi += 1
                for _ in range(2):
                    if ci < len(cu):
                        mix.append(cu[ci])
                        ci += 1
            return units + mix

        def run_attention(b, filler, depth):
            nslots = H + depth - 1
            per = [[] for _ in range(nslots)]
            n = len(filler)
            for idx, u in enumerate(filler):
                per[idx * nslots // max(n, 1)].append(u)
            for t in range(nslots):
                if t < H:
                    emit_scores(b, t)
                for u in per[t]:
                    u()
                if t >= depth - 1:
                    emit_pv(b, t - depth + 1)
            flush_norms()

        # prologue: batch 0 projections run dense
        for u in proj_units(0):
            u()
        for b in range(NB):
            filler = []
            if b + 1 < NB:
                filler += proj_units(b + 1)
            if b - 1 >= 0:
                filler += [
                    partial(unit_E, b - 1, i, n)
                    for i in range(QT)
                    for n in range(NCH)
                ]
            depth = 3 if b == NB - 1 else 2
            run_attention(b, filler, depth)
        # epilogue: last batch's output projection
        for i in range(QT):
            for n in range(NCH):
                unit_E(NB - 1, i, n)

    nc.compile()
    return nc


_NC_CACHE = None


def _get_nc():
    global _NC_CACHE
    if _NC_CACHE is None:
        _NC_CACHE = _build()
    return _NC_CACHE


def run(inputs, trace=False):
    if trace:
        os.environ.pop("BASS_NEVER_TRACE", None)
    else:
        # keep the spmd runner off the NTFF trace path (the profiling hook
        # module is not always present)
        os.environ["BASS_NEVER_TRACE"] = "1"
    # hidden_states and weights are pre-cast to fp16 on the host: identical
    # rounding to the on-chip cast, but half the DMA bytes and no staging
    hs = np.ascontiguousarray(
        np.asarray(inputs["hidden_states"], dtype=np.float32).astype(np.float16)
    )
    assert hs.shape == (B, S, E)
    wb = {}
    for nm in ("q", "k", "v", "o"):
        wb[f"W{nm}"] = np.ascontiguousarray(
            np.asarray(inputs[f"W{nm}"], dtype=np.float32).astype(np.float16)
        )
        wb[f"b{nm}"] = np.ascontiguousarray(
            np.asarray(inputs[f"b{nm}"], dtype=np.float32)
        )

    nc = _get_nc()
    in_maps = []
    for c in range(NCORES):
        m = {"hs": hs[c * NB:(c + 1) * NB]}
        m.update(wb)
        in_maps.append(m)
    res = run_bass_kernel_spmd(
        nc, in_maps, core_ids=list(range(NCORES)), trace=trace
    )
    outp = np.concatenate([r_["out"] for r_ in res.results], axis=0)
    return outp, res


def kernel(**inputs) -> np.ndarray:
    # retry once on transient accelerator errors (rare NRT exec glitches)
    last = None
    for attempt in range(2):
        try:
            outp, _ = run(inputs, trace=False)
            return outp
        except Exception as e:  # noqa: BLE001
            last = e
            time.sleep(10)
    raise last
